# revision 17
# baseline (speedup 1.0000x reference)
"""Trainium2 Bass kernel for nn_DifferentiableMPO_cvx (batched simplex-QP FISTA).

Math (per batch b): 300 FISTA iterations of
    w <- proj_simplex(y - step * (2*Sigma_{b,h} y + 2C*chain(y)))
with Sigma = L L^T per (b,h), step = 1/(2*max_h ||Sigma||_F + 8C).

Strategy:
- Pure data parallel over B=128 across 8 cores (16 per core, 192 (b,h) blocks).
- Sigma precomputed on PE (fp32), split into bf16 hi+lo halves kept in SBUF.
- Per-iteration matvec: 2 accumulating bf16 matmuls per block over a packed
  [y_hi, y_lo] rhs pair -> full 4-term split product, fp32 PSUM accumulate
  (trajectory matches fp32 reference to ~1e-5 rel).
- Simplex projection by warm-started Newton on the dual threshold theta
  (exact after ~2 steps; we do 3), using fused tensor_scalar ops with
  accum_out for the partition-local sums.
- State layouts: [N=128 part, q=192 free] (t-major columns q = h*16+b) for
  matvec/chain; [q part (2x96), N free] for projection/momentum; PE-transposes
  bridge the two.
"""
import numpy as np

import concourse.bass as bass
import concourse.bacc as bacc
import concourse.tile as tile
import concourse.mybir as mybir
from concourse import bass_utils
from concourse.masks import make_identity

B, H, N = 128, 12, 128
NCORES = 8
BS = B // NCORES          # 16 batch elements per core
NBH = BS * H              # 192 blocks per core
C = 0.01
N_ITERS = 300
NEWTON_STEPS = 3

f32 = mybir.dt.float32
bf16 = mybir.dt.bfloat16
Alu = mybir.AluOpType


def build(n_iters=N_ITERS):
    nc = bacc.Bacc("TRN2", target_bir_lowering=False, debug=False,
                   enable_asserts=False, num_devices=1)
    L_d = nc.dram_tensor("L", [BS, H, N, N], f32, kind="ExternalInput").ap()
    wp_d = nc.dram_tensor("w_prev", [BS, N], f32, kind="ExternalInput").ap()
    out_d = nc.dram_tensor("w_out", [BS, H, N], f32, kind="ExternalOutput").ap()

    with tile.TileContext(nc) as tc:
        with tc.tile_pool(name="consts", bufs=1) as consts, \
             tc.tile_pool(name="state", bufs=1) as state, \
             tc.tile_pool(name="lpsum", bufs=1, space="PSUM") as lpsum:

            ident = consts.tile([N, N], f32)
            make_identity(nc, ident)
            ones_col = consts.tile([N, 1], f32)
            nc.vector.memset(ones_col, 1.0)
            ones_row = consts.tile([1, N], f32)
            nc.vector.memset(ones_row, 1.0)

            wp_sb = consts.tile([BS, N], f32)
            nc.sync.dma_start(out=wp_sb, in_=wp_d)
            # w_prev^T: [N, BS]
            wpT = consts.tile([N, BS], f32)

            # big Sigma stores (bf16 hi/lo), block q at cols q*N:(q+1)*N
            Sh = consts.tile([N, NBH * N], bf16)
            Sl = consts.tile([N, NBH * N], bf16)

            # fro accumulation
            FroP = consts.tile([N, NBH], f32)

            # per-column -2*step and -2*C*step broadcast tiles
            nstep2T = consts.tile([N, NBH], f32)
            ncsT = consts.tile([N, NBH], f32)

            # ---------------- Sigma phase ----------------
            with tc.tile_pool(name="sig_sb", bufs=3) as sig_sb, \
                 tc.tile_pool(name="sig_ps", bufs=1, space="PSUM") as sig_ps:
                ps_wp = sig_ps.tile([N, NBH], f32, tag="ps_misc", name="ps_wp")
                nc.tensor.transpose(ps_wp[:, 0:BS], wp_sb, ident[0:BS, 0:BS])
                nc.scalar.copy(out=wpT, in_=ps_wp[:, 0:BS])

                for q in range(NBH):
                    h, b = q // BS, q % BS
                    l_sb = sig_sb.tile([N, N], f32, tag="l_sb")
                    nc.sync.dma_start(out=l_sb, in_=L_d[b, h])
                    ps_lt = sig_ps.tile([N, N], f32, tag="ps_lt", bufs=2)
                    nc.tensor.transpose(ps_lt, l_sb, ident)
                    lt_sb = sig_sb.tile([N, N], f32, tag="lt_sb")
                    nc.scalar.copy(out=lt_sb, in_=ps_lt)
                    ps_sig = sig_ps.tile([N, N], f32, tag="ps_sig", bufs=2)
                    nc.tensor.matmul(ps_sig, lt_sb, lt_sb)
                    # split into bf16 hi + lo
                    blk = slice(q * N, (q + 1) * N)
                    nc.scalar.copy(out=Sh[:, blk], in_=ps_sig)
                    nc.vector.tensor_sub(Sl[:, blk], ps_sig, Sh[:, blk])
                    # fro: sum of squares
                    sq_sb = sig_sb.tile([N, N], f32, tag="sq_sb")
                    nc.scalar.square(out=sq_sb, in_=ps_sig)
                    nc.vector.tensor_reduce(
                        FroP[:, q:q + 1], sq_sb, axis=mybir.AxisListType.X,
                        op=Alu.add)

                # fro2[1, q] = sum_p FroP[p, q]
                ps_f = sig_ps.tile([1, NBH], f32, tag="ps_misc", name="ps_f")
                nc.tensor.matmul(ps_f, ones_col, FroP)
                fro_row = sig_sb.tile([1, NBH], f32, tag="fro_row")
                nc.scalar.sqrt(out=fro_row, in_=ps_f)
                # max over h for fixed b: view [1, (b:16 stride 1), (h:12 stride 16)]
                fro_v = fro_row[:].rearrange("o (h b) -> o b h", b=BS)
                maxf = sig_sb.tile([1, BS], f32, tag="maxf")
                nc.vector.tensor_reduce(maxf, fro_v, axis=mybir.AxisListType.X,
                                        op=Alu.max)
                # Lf = 2*maxf + 8C ; step = 1/Lf
                lf = sig_sb.tile([1, BS], f32, tag="lf")
                nc.vector.tensor_scalar(out=lf, in0=maxf, scalar1=2.0,
                                        scalar2=8.0 * C, op0=Alu.mult,
                                        op1=Alu.add)
                step_row = sig_sb.tile([1, BS], f32, tag="step_row")
                nc.vector.reciprocal(out=step_row, in_=lf)
                ns2_row = sig_sb.tile([1, BS], f32, tag="ns2_row")
                nc.vector.tensor_scalar_mul(ns2_row, step_row, -2.0)
                ncs_row = sig_sb.tile([1, BS], f32, tag="ncs_row")
                nc.vector.tensor_scalar_mul(ncs_row, step_row, -2.0 * C)
                # repeat 12x along h -> [1, 192]
                ns2_192 = sig_sb.tile([1, NBH], f32, tag="ns2_192")
                ncs_192 = sig_sb.tile([1, NBH], f32, tag="ncs_192")
                for r_out, r_in in ((ns2_192, ns2_row), (ncs_192, ncs_row)):
                    for h in range(H):
                        nc.vector.tensor_copy(r_out[:, BS * h:BS * (h + 1)],
                                              r_in)
                # broadcast down partitions via K=1 matmul
                ps_b1 = sig_ps.tile([N, NBH], f32, tag="ps_misc", name="ps_b1")
                nc.tensor.matmul(ps_b1, ones_row, ns2_192)
                nc.scalar.copy(out=nstep2T, in_=ps_b1)
                ps_b2 = sig_ps.tile([N, NBH], f32, tag="ps_misc", name="ps_b2")
                nc.tensor.matmul(ps_b2, ones_row, ncs_192)
                nc.scalar.copy(out=ncsT, in_=ps_b2)

            # ---------------- state init ----------------
            def t2(shape, dt, name):
                return [state.tile(shape, dt, name=f"{name}{i}",
                                   tag=f"{name}{i}")
                        for i in range(2)]

            yT = state.tile([N, NBH], f32, tag="yT")
            yhl = state.tile([N, 2 * NBH], bf16, tag="yhl")
            zer96 = state.tile([96, N], f32, tag="zer96")
            nc.vector.memset(zer96, 0.0)
            wA = t2([96, N], f32, "wA")
            wB = t2([96, N], f32, "wB")
            z_sb = state.tile([N, NBH], f32, tag="z_sb")
            c1 = state.tile([N, NBH], f32, tag="c1")
            c2 = state.tile([N, NBH], f32, tag="c2")
            v1 = state.tile([N, NBH], f32, tag="v1")
            z1t = state.tile([N, NBH], f32, tag="z1t")
            v2 = state.tile([N, NBH], f32, tag="v2")
            Gsb = state.tile([N, NBH], f32, tag="Gsb")
            G2 = state.tile([N, 2 * NBH], f32, tag="G2")
            zh = t2([96, N], f32, "zh")
            relu_s = t2([96, N], f32, "relu_s")
            ind_s = t2([96, N], f32, "ind_s")
            tmp_m = t2([96, N], f32, "tmp_m")
            yh_half = t2([96, N], f32, "yh_half")
            th = t2([96, 1], f32, "th")
            ssum = t2([96, 1], f32, "ssum")
            cnt = t2([96, 1], f32, "cnt")
            dlt = t2([96, 1], f32, "dlt")
            rcn = t2([96, 1], f32, "rcn")
            # momentum scalars, replicated on all 128 partitions
            t_t = state.tile([N, 1], f32, tag="t_t")
            t2_t = state.tile([N, 1], f32, tag="t2_t")
            q4_t = state.tile([N, 1], f32, tag="q4_t")
            rt_t = state.tile([N, 1], f32, tag="rt_t")
            tn_t = state.tile([N, 1], f32, tag="tn_t")
            tm1_t = state.tile([N, 1], f32, tag="tm1_t")
            rtn_t = state.tile([N, 1], f32, tag="rtn_t")
            m_t = state.tile([N, 1], f32, tag="m_t")
            m1p_t = state.tile([N, 1], f32, tag="m1p_t")
            nm_t = state.tile([N, 1], f32, tag="nm_t")

            ps_g = lpsum.tile([N, 2 * NBH], f32, tag="ps_g")
            ps_zz = lpsum.tile([96, 2 * N], f32, tag="ps_zz")
            ps_z = [ps_zz[:, 0:N], ps_zz[:, N:2 * N]]
            ps_y = lpsum.tile([N, NBH], f32, tag="ps_y")

            nc.vector.memset(t_t, 1.0)
            for half in range(2):
                nc.vector.memset(th[half], 0.0)
                # w0 = broadcast of w_prev over t (6 groups of 16 rows per half)
                for t6 in range(6):
                    nc.sync.dma_start(out=wA[half][16 * t6:16 * (t6 + 1), :],
                                      in_=wp_sb)
            for h in range(H):
                nc.scalar.copy(out=yT[:, BS * h:BS * (h + 1)], in_=wpT)

            def iteration(w_in, w_out):
                # ---- pack y into [y_hi, y_lo] bf16 pairs ----
                ev = yhl[:, 0::2]
                od = yhl[:, 1::2]
                nc.vector.tensor_copy(ev, yT)
                nc.vector.tensor_sub(od, yT, ev)
                # ---- matvec: G = (Sh+Sl)(yh+yl), fp32 accumulated ----
                for q in range(NBH):
                    blk = slice(q * N, (q + 1) * N)
                    pr = slice(2 * q, 2 * q + 2)
                    nc.tensor.matmul(ps_g[:, pr], Sh[:, blk], yhl[:, pr],
                                     start=True, stop=False)
                    nc.tensor.matmul(ps_g[:, pr], Sl[:, blk], yhl[:, pr],
                                     start=False, stop=True)
                nc.scalar.copy(out=G2, in_=ps_g)
                nc.vector.tensor_add(Gsb, G2[:, 0::2], G2[:, 1::2])
                # ---- chain: c2 = 2y - y_prev - y_next (with boundaries) ----
                nc.vector.scalar_tensor_tensor(
                    out=c1[:, BS:], in0=yT[:, BS:], scalar=2.0,
                    in1=yT[:, :NBH - BS], op0=Alu.mult, op1=Alu.subtract)
                nc.vector.scalar_tensor_tensor(
                    out=c1[:, :BS], in0=yT[:, :BS], scalar=2.0,
                    in1=wpT, op0=Alu.mult, op1=Alu.subtract)
                nc.vector.tensor_sub(c2[:, :NBH - BS], c1[:, :NBH - BS],
                                     yT[:, BS:])
                nc.vector.tensor_sub(c2[:, NBH - BS:], c1[:, NBH - BS:],
                                     yT[:, NBH - BS:])
                # ---- z = y - 2*step*G - 2*C*step*c2 ----
                nc.vector.tensor_mul(v1, Gsb, nstep2T)
                nc.vector.tensor_add(z1t, yT, v1)
                nc.vector.tensor_mul(v2, c2, ncsT)
                nc.vector.tensor_add(z_sb, z1t, v2)
                # ---- transpose z to [q, N] halves ----
                for half in range(2):
                    cols = slice(96 * half, 96 * (half + 1))
                    nc.tensor.transpose(ps_z[half], z_sb[:, cols], ident)
                    nc.scalar.copy(out=zh[half], in_=ps_z[half])
                # ---- Newton on theta (warm started) ----
                for it_n in range(NEWTON_STEPS):
                    for half in range(2):
                        nc.vector.scalar_tensor_tensor(
                            out=relu_s[half], in0=zh[half], scalar=th[half],
                            in1=zer96, op0=Alu.subtract, op1=Alu.max,
                            accum_out=ssum[half])
                        nc.vector.scalar_tensor_tensor(
                            out=ind_s[half], in0=zh[half], scalar=th[half],
                            in1=zer96, op0=Alu.is_gt, op1=Alu.max,
                            accum_out=cnt[half])
                        nc.vector.tensor_scalar_max(cnt[half], cnt[half], 1.0)
                        nc.vector.tensor_scalar_add(dlt[half], ssum[half], -1.0)
                        nc.vector.reciprocal(out=rcn[half], in_=cnt[half])
                        nc.vector.scalar_tensor_tensor(
                            out=th[half], in0=dlt[half], scalar=rcn[half],
                            in1=th[half], op0=Alu.mult, op1=Alu.add)
                # ---- w_new = relu(z - theta); momentum ----
                # t-sequence update (all partitions compute the same values)
                nc.vector.tensor_mul(t2_t, t_t, t_t)
                nc.vector.tensor_scalar(out=q4_t, in0=t2_t, scalar1=4.0,
                                        scalar2=1.0, op0=Alu.mult, op1=Alu.add)
                nc.scalar.sqrt(out=rt_t, in_=q4_t)
                nc.vector.tensor_scalar(out=tn_t, in0=rt_t, scalar1=0.5,
                                        scalar2=0.5, op0=Alu.mult, op1=Alu.add)
                nc.vector.tensor_scalar_add(tm1_t, t_t, -1.0)
                nc.vector.reciprocal(out=rtn_t, in_=tn_t)
                nc.vector.tensor_mul(m_t, tm1_t, rtn_t)
                nc.vector.tensor_scalar_add(m1p_t, m_t, 1.0)
                nc.vector.tensor_scalar_mul(nm_t, m_t, -1.0)
                nc.vector.tensor_copy(t_t, tn_t)
                for half in range(2):
                    nc.vector.tensor_scalar(
                        out=w_out[half], in0=zh[half], scalar1=th[half],
                        scalar2=0.0, op0=Alu.subtract, op1=Alu.max)
                    # y = (1+m)*w_new - m*w_old
                    nc.vector.tensor_scalar_mul(tmp_m[half], w_in[half],
                                                nm_t[0:96, :])
                    nc.vector.scalar_tensor_tensor(
                        out=yh_half[half], in0=w_out[half],
                        scalar=m1p_t[0:96, :], in1=tmp_m[half],
                        op0=Alu.mult, op1=Alu.add)
                    # transpose back into yT columns
                    cols = slice(96 * half, 96 * (half + 1))
                    nc.tensor.transpose(ps_y[:, cols], yh_half[half],
                                        ident[0:96, 0:96])
                nc.scalar.copy(out=yT, in_=ps_y)

            if n_iters == -1:  # debug: single straight-line iteration
                iteration(wA, wB)
            else:
                with tc.For_i(0, n_iters, 2,
                              hint_engines=(mybir.EngineType.PE,)):
                    iteration(wA, wB)
                    iteration(wB, wA)

            # ---------------- output ----------------
            for h in range(H):
                half, t6 = divmod(h, 6)
                nc.sync.dma_start(
                    out=out_d[:, h, :],
                    in_=wA[half][16 * t6:16 * (t6 + 1), :])

    nc.compile()
    return nc


_NC = None


def kernel(mu, L, w_prev):
    global _NC
    if _NC is None:
        _NC = build()
    L = np.ascontiguousarray(L, dtype=np.float32)
    w_prev = np.ascontiguousarray(w_prev, dtype=np.float32)
    in_maps = []
    for c in range(NCORES):
        sl = slice(c * BS, (c + 1) * BS)
        in_maps.append({"L": L[sl], "w_prev": w_prev[sl]})
    res = bass_utils.run_bass_kernel_spmd(_NC, in_maps,
                                          core_ids=list(range(NCORES)))
    return np.concatenate([res.results[c]["w_out"] for c in range(NCORES)],
                          axis=0)
